# revision 30
# baseline (speedup 1.0000x reference)
"""BirthDeathIntervalLoss on 8 Trainium2 NeuronCores.

Strategy: the loss reads only 2*B*C*N*2 = 32768 scattered elements of the
512x512 prediction maps (4096 per core, data-parallel over batch). The
SWDGE indirect-DMA path moves one 4-byte descriptor per value at a fixed
aggregate rate (~4 ns/descriptor), so the kernel is structured to keep
that pipe busy from the earliest possible moment and to hide everything
else under it:
  1. flat gather offsets are fully precomputed on the host and arrive in
     one 16KB DMA (no on-device index math before the gather can start),
  2. 8 indirect gather calls (ascending sizes: a small first call gets
     descriptors flowing early; descriptor generation of later calls
     hides under the transfers) land values in 8 partition rows, pairs
     adjacent: row k holds birth,death,birth,death,...
  3. per-row compute (d=birth-death, d*d*w, row-sum) is emitted per
     gather call and runs on vector/gpsimd as soon as that call's data
     lands, overlapping the remaining transfers; only the last row's
     short chain trails the final semaphore,
  4. a [8,1]x[8,1] matmul reduces across rows, + additive const, one
     scalar out per core. The host sums the 8 partials.

The masked-mean algebra of the reference folds into a constant per-(set,
class, n) weight plus an additive constant:
  loss = sum W[s,c,n] * (birth-death)^2 + B * sum_s a_s*BETA*cnt_s/C
  W[s,c,n] = a_s * (-BETA/good_s[c] if n < good_s[c] else (1-BETA)/(N-good_s[c])) / C
with a_0 = ALPHA, a_1 = 1-ALPHA, cnt_s = #{c : good_s[c] > 0}.
"""

import numpy as np

# ---- problem constants (hardcoded per harness contract) ----
B, C, H, W, N = 32, 4, 512, 512, 64
GOOD = np.array([[1, 2, 1, 3], [1, 0, 2, 1]], dtype=np.int64)  # [set, class]
ALPHA = 0.5
BETA = 0.5
N_CORES = 8
B_LOC = B // N_CORES  # 4 batches per core

PRED_LOC = B_LOC * C * H * W          # 4,194,304 f32 per core
N_VALS = 2 * B_LOC * C * N * 2        # 4096 gathered values per core
N_PAIRS = N_VALS // 2                 # 2048 (birth,death) pairs per core

P = 128                               # offset-tile partitions
FV = N_VALS // P                      # 32 offset columns

# gather call sizes (descriptors, multiples of 128). Descending: each
# call's transfers must outlast the next call's serial descriptor
# generation or the transfer pipe starves; the tiny last call drains
# quickly after the final generation slice.
SIZES = [640, 640, 640, 640, 512, 512, 384, 128]
assert sum(SIZES) == N_VALS and all(s % P == 0 for s in SIZES)
KG = len(SIZES)
COLS = [s // P for s in SIZES]        # offset columns per call
CSTART = np.cumsum([0] + COLS)        # column offsets
FMAX = max(SIZES)                     # padded row length of g1
QP = FMAX // 2                        # pairs per padded row
PAIR_BASE = np.cumsum([0] + [s // 2 for s in SIZES])


def _host_constants():
    """Per-pair weights w[m] (natural order) and the per-core const."""
    a = np.array([ALPHA, 1.0 - ALPHA])
    m = np.arange(N_PAIRS)
    s = m // (B_LOC * C * N)
    cc = (m // N) % C
    n = m % N
    g = GOOD[s, cc]
    w = np.where(
        n < g,
        -a[s] * BETA / np.maximum(g, 1) / C,
        a[s] * (1.0 - BETA) / (N - g) / C,
    ).astype(np.float32)

    cnt = (GOOD > 0).sum(axis=1)  # per set
    const_per_core = float((a * BETA * cnt / C).sum() * B_LOC)
    return w, const_per_core


_WPAIR, _CONST = _host_constants()

# sort pairs by birth address so each DMA engine's descriptor stream has
# ascending addresses (HBM row-buffer locality); weights follow the
# permutation, so this is a host-side relabeling only
SORT_PAIRS = True

# image base of pair m's (b, c) plane within the core's flat pred block
_M = np.arange(N_PAIRS)
_BASE_PAIR = (
    (((_M // (C * N)) % B_LOC) * C + (_M // N) % C) * (H * W)
).astype(np.int64)

_PROGRAM = None
_LAST_RESULTS = None  # BassKernelResults of the most recent run (for test.py)
TRACE = False


def _build_program():
    from concourse import bacc, mybir
    import concourse.bass as bass
    import concourse.tile as tile

    f32 = mybir.dt.float32
    i32 = mybir.dt.int32

    nc = bacc.Bacc("TRN2", target_bir_lowering=False, debug=False)

    pred_d = nc.dram_tensor("pred", [PRED_LOC], f32, kind="ExternalInput")
    off_d = nc.dram_tensor("off", [P, FV], i32, kind="ExternalInput")
    wts_d = nc.dram_tensor("wts", [KG, QP], f32, kind="ExternalInput")
    out_d = nc.dram_tensor("out", [1, 1], f32, kind="ExternalOutput")

    with tile.TileContext(nc) as tc:
        with (
            tc.tile_pool(name="sb", bufs=1) as pool,
            tc.tile_pool(name="ps", bufs=1, space="PSUM") as psp,
        ):
            ones = pool.tile([KG, 1], f32)
            nc.vector.memset(ones[:], 1.0)

            off = pool.tile([P, FV], i32)
            nc.sync.dma_start(off[:], off_d[:])
            wts = pool.tile([KG, QP], f32)
            nc.scalar.dma_start(wts[:], wts_d[:])

            # row k gets call k's values: birth,death,birth,death,...
            # (one 4-byte descriptor per value; offsets consumed
            # partition-fastest from the call's column range). Row tails
            # keep garbage; their weights are zero. No memset: it would
            # add a second semaphore wait in front of the first gather.
            g1 = pool.tile([KG, FMAX], f32)
            src = pred_d.ap().rearrange("(a f) -> a f", a=1)
            for k in range(KG):
                c0, c1 = int(CSTART[k]), int(CSTART[k + 1])
                nc.gpsimd.indirect_dma_start(
                    out=g1[k : k + 1, 0 : SIZES[k]].rearrange(
                        "a (f one) -> a f one", one=1
                    ),
                    out_offset=None,
                    in_=src,
                    in_offset=bass.IndirectOffsetOnAxis(
                        ap=off[:, c0:c1], axis=1
                    ),
                )

            # one combined chain over all 8 rows: DVE op cost is
            # independent of partition count, so splitting into groups
            # only multiplies the fixed per-op cost
            d = pool.tile([KG, QP], f32)
            nc.vector.tensor_tensor(
                out=d[:], in0=g1[:, 0 : 2 * QP : 2], in1=g1[:, 1 : 2 * QP : 2],
                op=mybir.AluOpType.subtract,
            )
            dw = pool.tile([KG, QP], f32)
            nc.vector.tensor_tensor(
                out=dw[:], in0=d[:], in1=wts[:], op=mybir.AluOpType.mult
            )
            dw2 = pool.tile([KG, QP], f32)
            nc.vector.tensor_tensor(
                out=dw2[:], in0=dw[:], in1=d[:], op=mybir.AluOpType.mult
            )
            r = pool.tile([KG, 1], f32)
            nc.vector.reduce_sum(out=r[:], in_=dw2[:], axis=mybir.AxisListType.X)

            acc = psp.tile([1, 1], f32)
            nc.tensor.matmul(acc[:], lhsT=r[:], rhs=ones[:], start=True, stop=True)
            res = pool.tile([1, 1], f32)
            nc.vector.tensor_scalar(
                out=res[:],
                in0=acc[:],
                scalar1=_CONST,
                scalar2=None,
                op0=mybir.AluOpType.add,
            )
            nc.sync.dma_start(out_d[:], res[:])

    nc.compile()
    return nc


def _get_program():
    global _PROGRAM
    if _PROGRAM is None:
        _PROGRAM = _build_program()
    return _PROGRAM


def _pack_core(i0c, i1c):
    """Offset tile [P, FV] and weight map [KG, QP] for one core."""
    iv = np.stack([i0c, i1c])  # [2, B_LOC, C, N, 2, 2]
    flat = iv[..., 0] * W + iv[..., 1]  # [2, B_LOC, C, N, 2]
    birth = flat[..., 0].reshape(N_PAIRS) + _BASE_PAIR
    death = flat[..., 1].reshape(N_PAIRS) + _BASE_PAIR
    w = _WPAIR
    if SORT_PAIRS:
        order = np.argsort(birth, kind="stable")
        birth, death, w = birth[order], death[order], w[order]
    # value sequence: call k emits birth(m),death(m) for its pair range;
    # its j-th descriptor offset sits at off[j % P, CSTART[k] + j // P]
    off = np.empty((P, FV), dtype=np.int32)
    wts = np.zeros((KG, QP), dtype=np.float32)
    for k in range(KG):
        m0, m1 = int(PAIR_BASE[k]), int(PAIR_BASE[k + 1])
        seq = np.empty(SIZES[k], dtype=np.int64)
        seq[0::2] = birth[m0:m1]
        seq[1::2] = death[m0:m1]
        c0, c1 = int(CSTART[k]), int(CSTART[k + 1])
        off[:, c0:c1] = seq.reshape(COLS[k], P).T
        wts[k, 0 : m1 - m0] = w[m0:m1]
    return off, wts


def kernel(prediction, intervals_comp_0, intervals_comp_1):
    global _LAST_RESULTS
    from concourse.bass_utils import run_bass_kernel_spmd

    nc = _get_program()

    prediction = np.asarray(prediction, dtype=np.float32)
    i0 = np.asarray(intervals_comp_0, dtype=np.int64)
    i1 = np.asarray(intervals_comp_1, dtype=np.int64)

    in_maps = []
    for mcore in range(N_CORES):
        sl = slice(mcore * B_LOC, (mcore + 1) * B_LOC)
        off, wts = _pack_core(i0[sl], i1[sl])
        in_maps.append(
            {
                "pred": np.ascontiguousarray(prediction[sl]).reshape(-1),
                "off": off,
                "wts": wts,
            }
        )

    results = run_bass_kernel_spmd(
        nc, in_maps, list(range(N_CORES)), trace=TRACE
    )
    _LAST_RESULTS = results
    total = sum(float(r["out"][0, 0]) for r in results.results)
    return np.array(total, dtype=np.float32)
